# revision 3
# baseline (speedup 1.0000x reference)
"""2D Haar DWT (level 1) Trainium2 Bass kernel — fp16 I/O.

Input  x: [16, 64, 256, 256] f32
Output y: [16, 256, 128, 128] f32, y[n, s*64+c, i, j] = Haar mix s of the
2x2 block x[n, c, 2i:2i+2, 2j:2j+2].

Sharding: pure data parallel over the batch dim — core k gets batches
[2k, 2k+2).

The transform is pure data movement (out bytes == in bytes), so the kernel
is HBM-bound: per-core f32 traffic would be 67 MB (~187 us at the 358 GB/s
per-NC HBM limit). The rel-err budget (2e-2) admits fp16, halving traffic
to 33.5 MB/core (~94 us roofline). The host:
  - scales x by 0.5 (exact power of two — folds the whole Haar
    normalization, so the device does pure +/- butterflies),
  - casts to fp16,
  - de-interleaves even/odd columns to [n, c, h, 2, 128] so BOTH device
    butterfly stages are unit-stride (DVE 2x_1P perf mode needs 16-bit
    dtype + step 1 + 4B alignment; a stride-2 stage would run 1x and make
    DVE the bottleneck at ~105 us/core),
  - upcasts the fp16 result to f32 on the way out.

Per-core device pipeline, G=32 channels per group (4 groups of 4 MB):
  load  x[n, c0:c0+32]  -> it[p=(c,q), (o t j)]  one contiguous 4 MB DMA
                           (p = c*4+q holds rows [64q, 64q+64) of channel c)
  stage1 (vertical):     sdv[:,0/1] = rows 2r +/- 2r+1     (2 DVE ops, 2x)
  stage2 (horizontal):   oadd/osub  = t=0 +/- t=1 columns  (2 DVE ops, 2x)
  store  4 subband slices y[n, s*64+c0 : +32] — each one contiguous 1 MB
         DMA (partition p maps to output rows [32q, 32q+32) of channel c).
Loads ride the sync HWDGE ring, stores the scalar ring, so loads never
queue behind stores. DVE ~69 us/core sits under the ~94 us DMA roofline.
"""

import sys

sys.path.insert(0, "/opt/trn_rl_repo")

import numpy as np

import concourse.bacc as bacc
import concourse.mybir as mybir
from concourse.tile import TileContext

N_CORES = 8
N_PER_CORE = 2  # batches per core
C = 64  # input channels
H = 256
W = 256
W2 = W // 2
G = 32  # channels per group (4 MB loads)
F16 = mybir.dt.float16


def build_nc():
    nc = bacc.Bacc("TRN2", target_bir_lowering=False, debug=False)
    x = nc.dram_tensor("x", [N_PER_CORE, C, H, 2, W2], F16, kind="ExternalInput")
    y = nc.dram_tensor("y", [N_PER_CORE, 4 * C, H // 2, W2], F16, kind="ExternalOutput")

    with TileContext(nc) as tc:
        with (
            tc.tile_pool(name="inpool", bufs=2) as inpool,
            tc.tile_pool(name="sdpool", bufs=2) as sdpool,
            tc.tile_pool(name="outpool", bufs=2) as outpool,
        ):
            for n in range(N_PER_CORE):
                for c0 in range(0, C, G):
                    # --- load: pure reshape of the 4 MB contiguous group.
                    # it[p, (o t j)] = x[n, c0 + p//4, 64*(p%4) + o, t, j]
                    it = inpool.tile([128, G * 512], F16, tag="in")
                    src = x[n, c0 : c0 + G].rearrange(
                        "c (q o) t j -> (c q) (o t j)", q=4
                    )
                    nc.sync.dma_start(out=it[:], in_=src)

                    # --- stage 1 (vertical): rows 2r / 2r+1 within a partition
                    itv = it[:].rearrange("p (r u f) -> p r u f", r=G, u=2)
                    sd = sdpool.tile([128, G * 512], F16, tag="sd")
                    sdv = sd[:].rearrange("p (v r f) -> p v r f", v=2, r=G)
                    nc.vector.tensor_add(
                        out=sdv[:, 0], in0=itv[:, :, 0], in1=itv[:, :, 1]
                    )
                    nc.vector.tensor_sub(
                        out=sdv[:, 1], in0=itv[:, :, 0], in1=itv[:, :, 1]
                    )

                    # --- stage 2 (horizontal): even/odd column planes (both
                    # unit-stride thanks to the host de-interleave)
                    sdt = sd[:].rearrange("p (w t j) -> p w t j", t=2, j=W2)
                    oadd = outpool.tile([128, G * 256], F16, tag="oadd")
                    osub = outpool.tile([128, G * 256], F16, tag="osub")
                    oav = oadd[:].rearrange("p (w j) -> p w j", j=W2)
                    osv = osub[:].rearrange("p (w j) -> p w j", j=W2)
                    nc.vector.tensor_add(out=oav, in0=sdt[:, :, 0], in1=sdt[:, :, 1])
                    nc.vector.tensor_sub(out=osv, in0=sdt[:, :, 0], in1=sdt[:, :, 1])

                    # --- stores: oadd = subbands 0,1; osub = subbands 2,3.
                    # Each is one contiguous 1 MB DRAM slice.
                    for t_, v, s in ((oadd, 0, 0), (oadd, 1, 1), (osub, 0, 2), (osub, 1, 3)):
                        dst = y[n, s * C + c0 : s * C + c0 + G].rearrange(
                            "c (q r) j -> (c q) (r j)", q=4
                        )
                        nc.scalar.dma_start(
                            out=dst,
                            in_=t_[:].rearrange("p (v f) -> p v f", v=2)[:, v],
                        )

    nc.finalize()
    return nc


_NC = None


def _get_nc():
    global _NC
    if _NC is None:
        _NC = build_nc()
    return _NC


def _make_in_maps(x: np.ndarray) -> list[dict]:
    """Host prep: *0.5, cast fp16, de-interleave even/odd columns."""
    x = np.asarray(x)
    assert x.shape == (16, C, H, W), x.shape
    xr = x.reshape(16, C, H, W2, 2)
    xp = np.empty((16, C, H, 2, W2), dtype=np.float16)
    np.multiply(xr[..., 0], np.float32(0.5), out=xp[:, :, :, 0, :])
    np.multiply(xr[..., 1], np.float32(0.5), out=xp[:, :, :, 1, :])
    return [
        {"x": xp[k * N_PER_CORE : (k + 1) * N_PER_CORE]} for k in range(N_CORES)
    ]


def _gather(results: list[dict]) -> np.ndarray:
    y16 = np.concatenate([r["y"] for r in results], axis=0)
    return y16.astype(np.float32)


def kernel(x: np.ndarray) -> np.ndarray:
    from concourse.bass_utils import run_bass_kernel_spmd

    nc = _get_nc()
    in_maps = _make_in_maps(x)
    res = run_bass_kernel_spmd(nc, in_maps, core_ids=list(range(N_CORES)))
    return _gather(res.results)
